# revision 27
# baseline (speedup 1.0000x reference)
"""EMA head kernel for Trainium2 (Bass/Tile), 8 NeuronCores.

Problem: alpha = clip(sigmoid(MLP(feat)), 0.01, 0.99) per (t, b);
         y[0] = r[0]; y[t] = (1-alpha[t])*y[t-1] + alpha[t]*r[t].

Sharding: 2 batch-halves x 4 time-quarters.  Each core handles 128
b-lanes (full partition dim) x 1024 t.  Per-core affine-scan pieces
z/P per segment; host chains carries across segments/slabs with
y = z + P*carry (carry_0 = r[0] reproduces y[0] = r[0] exactly).

Math: W2, b1 and the layer-1 bias are folded away on the host:
  w1f[:, h'] = S * |w2[h']| * W1[:, h']   (S = 16, pos w2 first)
  feat' = feat + beta  where  w1f.T beta = S*|w2|*b1  (least-norm)
so  sum_h w2*relu(x+b1) = (sum_pos relu(y) - sum_neg relu(y))/S  with
y = feat' @ w1f.  The per-block epilogue is one ACT Relu (PSUM fp32
-> SBUF fp16) and two DVE reduces (pos/neg column groups into
apreP/apreN); a per-segment Pool subtract + ACT sigmoid(bias=b2,
scale=1/S) recovers alpha.

feat is fp8(e4m3), host-pretransposed to [f, t*128b]; everything
vector-side is fp16 (scan state is fp32 internally).  feat streams on
the sync queue; z/p issues are deferred behind the feat issues so
they can never stall the feat stream.
"""

import numpy as np

T, B, FEAT, HID = 4096, 256, 128, 16
NCORES = 8
TQ, BH = 4, 2        # time-quarters x batch-halves
TLOC = T // TQ       # 1024 t per core
BLOC = B // BH       # 128 b per core
S = 16.0             # w1 fold scale
CH_T = 128           # t-steps per feat chunk (2 MB fp8)
TBLK = 64            # t-steps per PSUM block (2 banks)
# scan/tail segment boundaries within a core's 1024 t (coarse early, fine
# at the end); scans restart at (0,1) per segment, host chains carries
SEGS = [0, 256, 512, 768, 896, 960, 1024]

_CACHE = {}


def _build_program(npos):
    import concourse.bacc as bacc
    import concourse.tile as tile
    from concourse import mybir

    fp32 = mybir.dt.float32
    fp16 = mybir.dt.float16
    fp8 = mybir.dt.float8e4
    AF = mybir.ActivationFunctionType
    OP = mybir.AluOpType

    nc = bacc.Bacc("TRN2", target_bir_lowering=False, debug=False,
                   num_devices=NCORES)

    feat_d = nc.dram_tensor("feat", [FEAT, TLOC * BLOC], fp8,
                            kind="ExternalInput")
    rt_d = nc.dram_tensor("rt", [BLOC, TLOC], fp16, kind="ExternalInput")
    w1_d = nc.dram_tensor("w1", [FEAT, HID], fp8, kind="ExternalInput")
    b2col_d = nc.dram_tensor("b2col", [BLOC, 1], fp32, kind="ExternalInput")
    z_d = nc.dram_tensor("z", [BLOC, TLOC], fp16, kind="ExternalOutput")
    p_d = nc.dram_tensor("p", [BLOC, TLOC], fp16, kind="ExternalOutput")

    with tile.TileContext(nc) as tc:
        with (
            tc.tile_pool(name="singles", bufs=1) as singles,
            tc.tile_pool(name="featin", bufs=6) as featin,
            tc.tile_pool(name="hps", bufs=3, space="PSUM") as hps,
            tc.tile_pool(name="hwork", bufs=3) as hwork,
        ):
            # first feat chunk DMA before anything else (shortest lead-in);
            # tapered sub-splits so the first 64-t block is ready earliest
            ft0 = featin.tile([128, CH_T * BLOC], fp8, tag="ft")
            for lo, hi in ((0, 16), (16, 32), (32, 64), (64, 128)):
                nc.sync.dma_start(ft0[:, lo * BLOC:hi * BLOC],
                                  feat_d[:, lo * BLOC:hi * BLOC])

            # constants / small inputs on the scalar queue
            w1_sb = singles.tile([128, HID], fp8)
            nc.scalar.dma_start(w1_sb, w1_d[:, :])
            b2col = singles.tile([128, 1], fp32)
            nc.scalar.dma_start(b2col, b2col_d[:, :])
            rt_sb = singles.tile([128, TLOC], fp16)
            nc.scalar.dma_start(rt_sb, rt_d[:, :])
            ones_sb = singles.tile([128, TLOC], fp16)
            nc.vector.memset(ones_sb, 1.0)

            apreP = singles.tile([128, TLOC], fp16, name="apreP")
            apreN = singles.tile([128, TLOC], fp16, name="apreN")
            if npos == HID:
                nc.vector.memset(apreN, 0.0)
            if npos == 0:
                nc.vector.memset(apreP, 0.0)
            alpha = singles.tile([128, TLOC], fp16, name="alpha")
            A_sb = singles.tile([128, TLOC], fp16, name="A")
            Bv = singles.tile([128, TLOC], fp16, name="Bv")
            z_sb = singles.tile([128, TLOC], fp16, name="z")
            p_sb = singles.tile([128, TLOC], fp16, name="p")

            zp_out = []  # deferred z/p DMAs (sync queue, after ft issues)

            def tail_segment(lo, hi):
                sl = slice(lo, hi)
                nc.vector.tensor_sub(apreP[:, sl], apreP[:, sl],
                                     apreN[:, sl])
                nc.scalar.activation(alpha[:, sl], apreP[:, sl], AF.Sigmoid,
                                     bias=b2col, scale=1.0 / S)
                nc.vector.tensor_scalar(alpha[:, sl], alpha[:, sl],
                                        0.01, 0.99, op0=OP.max, op1=OP.min)
                nc.gpsimd.tensor_scalar(A_sb[:, sl], alpha[:, sl],
                                        -1.0, 1.0, op0=OP.mult, op1=OP.add)
                nc.gpsimd.tensor_mul(Bv[:, sl], alpha[:, sl], rt_sb[:, sl])
                # scans restart at (0, 1) each segment: the host chains the
                # carry, so consecutive tails are independent on device
                nc.vector.tensor_tensor_scan(
                    z_sb[:, sl], A_sb[:, sl], Bv[:, sl], 0.0,
                    op0=OP.mult, op1=OP.add)
                nc.vector.tensor_tensor_scan(
                    p_sb[:, sl], A_sb[:, sl], ones_sb[:, sl], 1.0,
                    op0=OP.mult, op1=OP.mult)
                zp_out.append(sl)

            # 128-t chunks, tapering to 64 t at the end so less epilogue
            # work remains once the feat stream finishes
            chunk_t = [CH_T] * 7 + [TBLK] * 2
            offs = np.cumsum([0] + chunk_t).tolist()
            for k, (t_lo, ct) in enumerate(zip(offs[:-1], chunk_t)):
                if k == 0:
                    ft = ft0
                else:
                    ft = featin.tile([128, CH_T * BLOC], fp8, tag="ft")
                    nsub = 2 if ct == CH_T else 1
                    sub = ct * BLOC // nsub
                    for q in range(nsub):
                        nc.sync.dma_start(
                            ft[:, q * sub:(q + 1) * sub],
                            feat_d[:, t_lo * BLOC + q * sub:
                                   t_lo * BLOC + (q + 1) * sub])
                for blk in range(ct // TBLK):
                    hbank = hps.tile([128, TBLK, HID], fp32, name="hbank")
                    for tt in range(TBLK):
                        col = (blk * TBLK + tt) * BLOC
                        nc.tensor.matmul(
                            hbank[:, tt, :], ft[:, col:col + BLOC], w1_sb,
                            start=True, stop=True, skip_group_check=True)
                    # ACT applies relu while converting PSUM fp32 -> fp16;
                    # DVE then sums the pos / neg column groups.
                    hw = hwork.tile([128, TBLK, HID], fp16, tag="hw")
                    nc.scalar.activation(hw, hbank, AF.Relu)
                    t0 = t_lo + blk * TBLK
                    with nc.allow_low_precision(
                            "fp16 apre validated vs numpy, 16-elem sums"):
                        if npos > 0:
                            nc.vector.tensor_reduce(
                                apreP[:, t0:t0 + TBLK], hw[:, :, :npos],
                                axis=mybir.AxisListType.X, op=OP.add)
                        if npos < HID:
                            nc.vector.tensor_reduce(
                                apreN[:, t0:t0 + TBLK], hw[:, :, npos:],
                                axis=mybir.AxisListType.X, op=OP.add)
                # coarse 256-t tails early (less DVE overhead), fine
                # 128/64-t tails at the end (short post-DMA drain)
                tails = {2: (0, 256), 4: (256, 512), 6: (512, 768),
                         7: (768, 896), 8: (896, 960)}
                if k in tails:
                    tail_segment(*tails[k])
            # deferred z/p output DMAs: on the sync queue, after every feat
            # chunk issue so they can never stall the feat stream
            for sl in zp_out:
                nc.sync.dma_start(z_d[:, sl], z_sb[:, sl])
                nc.sync.dma_start(p_d[:, sl], p_sb[:, sl])
            tail_segment(offs[-2], offs[-1])
            sl = zp_out[-1]
            nc.sync.dma_start(z_d[:, sl], z_sb[:, sl])
            nc.sync.dma_start(p_d[:, sl], p_sb[:, sl])

    nc.finalize()
    return nc


def _get_program(npos):
    key = ("nc", npos)
    if key not in _CACHE:
        _CACHE[key] = _build_program(npos)
    return _CACHE[key]


def _host_prep(r, feat, W1, b1, W2, b2):
    import ml_dtypes
    W1 = np.asarray(W1, dtype=np.float32)
    b1 = np.asarray(b1, dtype=np.float32).reshape(HID)
    W2 = np.asarray(W2, dtype=np.float32).reshape(HID)
    b2 = float(np.asarray(b2, dtype=np.float32).reshape(1)[0])

    perm = np.argsort(W2 < 0, kind="stable")
    w2s, b1s = W2[perm], b1[perm]
    npos = int((w2s >= 0).sum())

    w1f8 = (S * np.abs(w2s)[None, :] * W1[:, perm]).astype(
        ml_dtypes.float8_e4m3)
    w1fq = w1f8.astype(np.float64)  # dequantized, for the bias solve
    d = (S * np.abs(w2s) * b1s).astype(np.float64)
    beta = np.linalg.lstsq(w1fq.T, d, rcond=None)[0].astype(np.float32)

    b2col = np.full((BLOC, 1), b2, dtype=np.float32)

    r2 = r[:, :, 0]
    in_maps = []
    BLT = 32  # t rows per transpose block (1 MB window)
    for c_id in range(NCORES):
        tq, hb = divmod(c_id, BH)
        tsl = slice(tq * TLOC, (tq + 1) * TLOC)
        bsl = slice(hb * BLOC, (hb + 1) * BLOC)
        fblk = feat[tsl, bsl, :]  # [1024, 128, 128] (t, b, f) fp32
        featT = np.empty((FEAT, TLOC * BLOC), np.float32)
        for j in range(0, TLOC, BLT):
            featT[:, j * BLOC:(j + BLT) * BLOC] = (
                fblk[j:j + BLT].reshape(BLT * BLOC, FEAT).T)
        featT += beta[:, None]
        featT = featT.astype(ml_dtypes.float8_e4m3)
        rt = np.ascontiguousarray(r2[tsl, bsl].T).astype(np.float16)
        in_maps.append({
            "feat": featT, "rt": rt,
            "w1": w1f8, "b2col": b2col,
        })
    return in_maps, npos


def kernel(r, feat, W1, b1, W2, b2, _run_kwargs=None, _return_results=False):
    from concourse.bass_utils import run_bass_kernel_spmd

    r = np.asarray(r, dtype=np.float32)
    feat = np.asarray(feat, dtype=np.float32)

    in_maps, npos = _host_prep(r, feat, W1, b1, W2, b2)
    nc = _get_program(npos)

    kw = _run_kwargs or {}
    res = run_bass_kernel_spmd(nc, in_maps, core_ids=list(range(NCORES)), **kw)

    # host stitch: y = z + P*carry per segment, carry chained across the
    # per-core segments and across t-quarters
    y = np.empty((T, B), dtype=np.float32)
    for hb in range(BH):
        bsl = slice(hb * BLOC, (hb + 1) * BLOC)
        carry = r[0, bsl, 0].astype(np.float32)
        for tq in range(TQ):
            c_id = tq * BH + hb
            zc = res.results[c_id]["z"].astype(np.float32).T  # [TLOC, BLOC]
            pc = res.results[c_id]["p"].astype(np.float32).T
            y_slab = np.empty((TLOC, BLOC), np.float32)
            for lo, hi in zip(SEGS[:-1], SEGS[1:]):
                y_slab[lo:hi] = zc[lo:hi] + pc[lo:hi] * carry[None, :]
                carry = y_slab[hi - 1]
            y[tq * TLOC:(tq + 1) * TLOC, bsl] = y_slab
    out = y[:, :, None]
    if _return_results:
        return out, res
    return out


# revision 29
# speedup vs baseline: 1.0214x; 1.0214x over previous
"""EMA head kernel for Trainium2 (Bass/Tile), 8 NeuronCores.

Problem: alpha = clip(sigmoid(MLP(feat)), 0.01, 0.99) per (t, b);
         y[0] = r[0]; y[t] = (1-alpha[t])*y[t-1] + alpha[t]*r[t].

Sharding: 2 batch-halves x 4 time-quarters.  Each core handles 128
b-lanes (full partition dim) x 1024 t.  Per-core affine-scan pieces
z/P per segment; host chains carries across segments/slabs with
y = z + P*carry (carry_0 = r[0] reproduces y[0] = r[0] exactly).

Math: W2, b1 and the layer-1 bias are folded away on the host:
  w1f[:, h'] = S * |w2[h']| * W1[:, h']   (S = 16, pos w2 first)
  feat' = feat + beta  where  w1f.T beta = S*|w2|*b1  (least-norm)
so  sum_h w2*relu(x+b1) = (sum_pos relu(y) - sum_neg relu(y))/S  with
y = feat' @ w1f.  The per-block epilogue is one ACT Relu (PSUM fp32
-> SBUF fp16) and two DVE reduces (pos/neg column groups into
apreP/apreN); a per-segment DVE subtract + ACT sigmoid(bias=b2,
scale=1/S) recovers alpha.

feat is fp8(e4m3), host-pretransposed to [f, t*128b]; everything
vector-side is fp16 (scan state is fp32 internally).  feat streams on
the sync queue; z/p issues are deferred behind the feat issues so
they can never stall the feat stream.
"""

import numpy as np

T, B, FEAT, HID = 4096, 256, 128, 16
NCORES = 8
TQ, BH = 4, 2        # time-quarters x batch-halves
TLOC = T // TQ       # 1024 t per core
BLOC = B // BH       # 128 b per core
S = 16.0             # w1 fold scale
CH_T = 128           # t-steps per feat chunk (2 MB fp8)
TBLK = 64            # t-steps per PSUM block (2 banks)
# scan/tail segment boundaries within a core's 1024 t (coarse early, fine
# at the end); scans restart at (0,1) per segment, host chains carries
SEGS = [0, 256, 512, 768, 896, 960, 1024]

_CACHE = {}


def _build_program(npos):
    import concourse.bacc as bacc
    import concourse.tile as tile
    from concourse import mybir

    fp32 = mybir.dt.float32
    fp16 = mybir.dt.float16
    fp8 = mybir.dt.float8e4
    AF = mybir.ActivationFunctionType
    OP = mybir.AluOpType

    nc = bacc.Bacc("TRN2", target_bir_lowering=False, debug=False,
                   num_devices=NCORES)

    feat_d = nc.dram_tensor("feat", [FEAT, TLOC * BLOC], fp8,
                            kind="ExternalInput")
    rt_d = nc.dram_tensor("rt", [BLOC, TLOC], fp16, kind="ExternalInput")
    w1_d = nc.dram_tensor("w1", [FEAT, HID], fp8, kind="ExternalInput")
    b2col_d = nc.dram_tensor("b2col", [BLOC, 1], fp32, kind="ExternalInput")
    z_d = nc.dram_tensor("z", [BLOC, TLOC], fp16, kind="ExternalOutput")
    p_d = nc.dram_tensor("p", [BLOC, TLOC], fp16, kind="ExternalOutput")

    with tile.TileContext(nc) as tc:
        with (
            tc.tile_pool(name="singles", bufs=1) as singles,
            tc.tile_pool(name="featin", bufs=5) as featin,
            tc.tile_pool(name="hps", bufs=3, space="PSUM") as hps,
            tc.tile_pool(name="hwork", bufs=3) as hwork,
        ):
            # first feat chunk DMA before anything else (shortest lead-in);
            # tapered sub-splits so the first 64-t block is ready earliest
            ft0 = featin.tile([128, CH_T * BLOC], fp8, tag="ft")
            for lo, hi in ((0, 16), (16, 32), (32, 64), (64, 128)):
                nc.sync.dma_start(ft0[:, lo * BLOC:hi * BLOC],
                                  feat_d[:, lo * BLOC:hi * BLOC])

            # constants / small inputs on the scalar queue
            w1_sb = singles.tile([128, HID], fp8)
            nc.scalar.dma_start(w1_sb, w1_d[:, :])
            b2col = singles.tile([128, 1], fp32)
            nc.scalar.dma_start(b2col, b2col_d[:, :])
            rt_sb = singles.tile([128, TLOC], fp16)
            nc.scalar.dma_start(rt_sb, rt_d[:, :])
            ones_sb = singles.tile([128, TLOC], fp16)
            nc.vector.memset(ones_sb, 1.0)

            apreP = singles.tile([128, TLOC], fp16, name="apreP")
            apreN = singles.tile([128, TLOC], fp16, name="apreN")
            if npos == HID:
                nc.vector.memset(apreN, 0.0)
            if npos == 0:
                nc.vector.memset(apreP, 0.0)
            alpha = singles.tile([128, TLOC], fp16, name="alpha")
            A_sb = singles.tile([128, TLOC], fp16, name="A")
            Bv = singles.tile([128, TLOC], fp16, name="Bv")
            z_sb = singles.tile([128, TLOC], fp16, name="z")
            p_sb = singles.tile([128, TLOC], fp16, name="p")

            zp_out = []  # deferred z/p DMAs (sync queue, after ft issues)

            def tail_segment(lo, hi):
                sl = slice(lo, hi)
                nc.vector.tensor_sub(apreP[:, sl], apreP[:, sl],
                                     apreN[:, sl])
                nc.scalar.activation(alpha[:, sl], apreP[:, sl], AF.Sigmoid,
                                     bias=b2col, scale=1.0 / S)
                nc.vector.tensor_scalar(alpha[:, sl], alpha[:, sl],
                                        0.01, 0.99, op0=OP.max, op1=OP.min)
                nc.gpsimd.tensor_scalar(A_sb[:, sl], alpha[:, sl],
                                        -1.0, 1.0, op0=OP.mult, op1=OP.add)
                nc.gpsimd.tensor_mul(Bv[:, sl], alpha[:, sl], rt_sb[:, sl])
                # scans restart at (0, 1) each segment: the host chains the
                # carry, so consecutive tails are independent on device
                nc.vector.tensor_tensor_scan(
                    z_sb[:, sl], A_sb[:, sl], Bv[:, sl], 0.0,
                    op0=OP.mult, op1=OP.add)
                nc.vector.tensor_tensor_scan(
                    p_sb[:, sl], A_sb[:, sl], ones_sb[:, sl], 1.0,
                    op0=OP.mult, op1=OP.mult)
                zp_out.append(sl)

            # 128-t chunks, tapering to 64 t at the end so less epilogue
            # work remains once the feat stream finishes
            chunk_t = [CH_T] * 7 + [TBLK] * 2
            offs = np.cumsum([0] + chunk_t).tolist()
            for k, (t_lo, ct) in enumerate(zip(offs[:-1], chunk_t)):
                if k == 0:
                    ft = ft0
                else:
                    ft = featin.tile([128, CH_T * BLOC], fp8, tag="ft")
                    nsub = 2 if ct == CH_T else 1
                    sub = ct * BLOC // nsub
                    for q in range(nsub):
                        nc.sync.dma_start(
                            ft[:, q * sub:(q + 1) * sub],
                            feat_d[:, t_lo * BLOC + q * sub:
                                   t_lo * BLOC + (q + 1) * sub])
                for blk in range(ct // TBLK):
                    hbank = hps.tile([128, TBLK, HID], fp32, name="hbank")
                    for tt in range(TBLK):
                        col = (blk * TBLK + tt) * BLOC
                        nc.tensor.matmul(
                            hbank[:, tt, :], ft[:, col:col + BLOC], w1_sb,
                            start=True, stop=True, skip_group_check=True)
                    # ACT applies relu while converting PSUM fp32 -> fp16;
                    # DVE then sums the pos / neg column groups.
                    hw = hwork.tile([128, TBLK, HID], fp16, tag="hw")
                    nc.scalar.activation(hw, hbank, AF.Relu)
                    t0 = t_lo + blk * TBLK
                    with nc.allow_low_precision(
                            "fp16 apre validated vs numpy, 16-elem sums"):
                        if npos > 0:
                            nc.vector.tensor_reduce(
                                apreP[:, t0:t0 + TBLK], hw[:, :, :npos],
                                axis=mybir.AxisListType.X, op=OP.add)
                        if npos < HID:
                            nc.vector.tensor_reduce(
                                apreN[:, t0:t0 + TBLK], hw[:, :, npos:],
                                axis=mybir.AxisListType.X, op=OP.add)
                # coarse 256-t tails early (less DVE overhead), fine
                # 128/64-t tails at the end (short post-DMA drain)
                tails = {2: (0, 256), 4: (256, 512), 6: (512, 768),
                         7: (768, 896), 8: (896, 960)}
                if k in tails:
                    tail_segment(*tails[k])
            # deferred z/p output DMAs: on the sync queue, after every feat
            # chunk issue so they can never stall the feat stream
            for sl in zp_out:
                nc.sync.dma_start(z_d[:, sl], z_sb[:, sl])
                nc.sync.dma_start(p_d[:, sl], p_sb[:, sl])
            tail_segment(offs[-2], offs[-1])
            sl = zp_out[-1]
            nc.sync.dma_start(z_d[:, sl], z_sb[:, sl])
            nc.sync.dma_start(p_d[:, sl], p_sb[:, sl])

    nc.finalize()
    return nc


def _get_program(npos):
    key = ("nc", npos)
    if key not in _CACHE:
        _CACHE[key] = _build_program(npos)
    return _CACHE[key]


def _host_prep(r, feat, W1, b1, W2, b2):
    import ml_dtypes
    W1 = np.asarray(W1, dtype=np.float32)
    b1 = np.asarray(b1, dtype=np.float32).reshape(HID)
    W2 = np.asarray(W2, dtype=np.float32).reshape(HID)
    b2 = float(np.asarray(b2, dtype=np.float32).reshape(1)[0])

    perm = np.argsort(W2 < 0, kind="stable")
    w2s, b1s = W2[perm], b1[perm]
    npos = int((w2s >= 0).sum())

    w1f8 = (S * np.abs(w2s)[None, :] * W1[:, perm]).astype(
        ml_dtypes.float8_e4m3)
    w1fq = w1f8.astype(np.float64)  # dequantized, for the bias solve
    d = (S * np.abs(w2s) * b1s).astype(np.float64)
    beta = np.linalg.lstsq(w1fq.T, d, rcond=None)[0].astype(np.float32)

    b2col = np.full((BLOC, 1), b2, dtype=np.float32)

    r2 = r[:, :, 0]
    in_maps = []
    BLT = 32  # t rows per transpose block (1 MB window)
    for c_id in range(NCORES):
        tq, hb = divmod(c_id, BH)
        tsl = slice(tq * TLOC, (tq + 1) * TLOC)
        bsl = slice(hb * BLOC, (hb + 1) * BLOC)
        fblk = feat[tsl, bsl, :]  # [1024, 128, 128] (t, b, f) fp32
        featT = np.empty((FEAT, TLOC * BLOC), np.float32)
        for j in range(0, TLOC, BLT):
            featT[:, j * BLOC:(j + BLT) * BLOC] = (
                fblk[j:j + BLT].reshape(BLT * BLOC, FEAT).T)
        featT += beta[:, None]
        featT = featT.astype(ml_dtypes.float8_e4m3)
        rt = np.ascontiguousarray(r2[tsl, bsl].T).astype(np.float16)
        in_maps.append({
            "feat": featT, "rt": rt,
            "w1": w1f8, "b2col": b2col,
        })
    return in_maps, npos


def kernel(r, feat, W1, b1, W2, b2, _run_kwargs=None, _return_results=False):
    from concourse.bass_utils import run_bass_kernel_spmd

    r = np.asarray(r, dtype=np.float32)
    feat = np.asarray(feat, dtype=np.float32)

    in_maps, npos = _host_prep(r, feat, W1, b1, W2, b2)
    nc = _get_program(npos)

    kw = _run_kwargs or {}
    res = run_bass_kernel_spmd(nc, in_maps, core_ids=list(range(NCORES)), **kw)

    # host stitch: y = z + P*carry per segment, carry chained across the
    # per-core segments and across t-quarters
    y = np.empty((T, B), dtype=np.float32)
    for hb in range(BH):
        bsl = slice(hb * BLOC, (hb + 1) * BLOC)
        carry = r[0, bsl, 0].astype(np.float32)
        for tq in range(TQ):
            c_id = tq * BH + hb
            zc = res.results[c_id]["z"].astype(np.float32).T  # [TLOC, BLOC]
            pc = res.results[c_id]["p"].astype(np.float32).T
            y_slab = np.empty((TLOC, BLOC), np.float32)
            for lo, hi in zip(SEGS[:-1], SEGS[1:]):
                y_slab[lo:hi] = zc[lo:hi] + pc[lo:hi] * carry[None, :]
                carry = y_slab[hi - 1]
            y[tq * TLOC:(tq + 1) * TLOC, bsl] = y_slab
    out = y[:, :, None]
    if _return_results:
        return out, res
    return out
